# revision 3
# baseline (speedup 1.0000x reference)
"""Trainium2 Bass kernel for the ANI (anisotropy) L1 loss.

Math (per voxel, per 3x3 symmetric tensor with channels xx,xy,xz,yy,yz,zz):
  y_c = gt_std[c] * x_c + gt_mean[c]            (affine; mask handled at the end)
  A   = [[y0,y1,y2],[y1,y3,y4],[y2,y4,y5]]
  q   = tr(A)/3 ;  C = A - q I
  p2  = ||C||_F^2 ;  p = sqrt(p2/6) ;  det = det(C)
  r   = det / (2 p^3)  in [-1,1]
  phi = arccos(r)/3
  eigenvalues l_k = q + 2 p cos(phi + k*2pi/3)
  ani(input)  = l2 - (l0+l1)/2 = 3 p cos(phi)
  ani(target) = (l0+l1)/2      = q - p cos(phi)
  loss = sum(|ani_in - ani_tg| * m) / max(sum(m), 1)

Key identities used on device (no arccos/cos/rsqrt in HW tables):
  cos(arccos(r)/3) = sin(pi/3 + arctan(w)/3),  w = r/sqrt(1-r^2)
  w = sqrt(54) * det / sqrt(g),  g = p2^3 - 54 det^2   (no normalization by p^3)

Sharding: pure data-parallel over the flattened spatial axis (8 cores).
Each core returns per-partition partial (masked |diff| sum, mask count);
the host reduces them to the scalar loss.
"""

import numpy as np

import concourse.bass as bass
import concourse.tile as tile
from concourse import bacc, mybir
from concourse.bass_utils import run_bass_kernel_spmd

F32 = mybir.dt.float32
I32 = mybir.dt.int32
ALU = mybir.AluOpType
AF = mybir.ActivationFunctionType

N_CORES = 8
B, C = 4, 6
HWD = 96 * 96 * 96          # 884736
SH = HWD // N_CORES         # 110592 spatial elems per core per (b, c)
P = 128
FREE = SH // P              # 864
NT = 432                    # free-dim elems per compute tile
NCH = FREE // NT            # chunks per b-plane

SQRT54 = float(np.sqrt(54.0))
GMIN = 1e-30                # clamp for g = p2^3 - 54 det^2 (keeps rsqrt finite)
PEPS = 1e-25                # bias inside sqrt for p (guards p2 == 0)
PI3 = float(np.pi / 3.0)

_CACHE = {}


def _build(reps: int = 1):
    """Build + compile the per-core module. reps>1 repeats the compute loop
    (same data) for timing-by-differences; output is then meaningless."""
    nc = bacc.Bacc("TRN2", target_bir_lowering=False, debug=False,
                   num_devices=N_CORES)
    x_in = nc.dram_tensor("input_data", [B, C, SH], F32, kind="ExternalInput")
    t_in = nc.dram_tensor("target", [B, C, SH], F32, kind="ExternalInput")
    m_in = nc.dram_tensor("mask", [B, SH], I32, kind="ExternalInput")
    sc_in = nc.dram_tensor("scal", [P, 16], F32, kind="ExternalInput")
    out = nc.dram_tensor("out", [P, 2], F32, kind="ExternalOutput")

    with tile.TileContext(nc) as tc:
        with (
            tc.tile_pool(name="const", bufs=1) as cpool,
            tc.tile_pool(name="io", bufs=2) as iopool,
            tc.tile_pool(name="tmp", bufs=1) as tpool,
            tc.tile_pool(name="acc", bufs=1) as apool,
            tc.tile_pool(name="part", bufs=2) as ppool,
        ):
            scal = cpool.tile([P, 16], F32, tag="scal")
            nc.sync.dma_start(scal[:], sc_in[:])

            lacc = apool.tile([P, 1], F32, tag="lacc")
            cacc = apool.tile([P, 1], F32, tag="cacc")
            nc.vector.memset(lacc[:], 0.0)
            nc.vector.memset(cacc[:], 0.0)

            def s_ap(c):   # gt_std[c] broadcast [P,1]
                return scal[:, c:c + 1]

            def mu_ap(c):  # gt_mean[c] broadcast [P,1]
                return scal[:, 6 + c:7 + c]

            peps_ap = scal[:, 12:13]   # PEPS
            pi3_ap = scal[:, 13:14]    # pi/3

            def tensor_chain(src, b, off, name):
                """Per-tensor chain up to (p2, det, rec, q). Returns dict of
                tiles; p2/det/rec stay alive for the trig stage."""
                xs = []
                for c in range(C):
                    xt = iopool.tile([P, NT], F32, tag=f"x{c}")
                    nc.sync.dma_start(
                        xt[:], src[b, c].rearrange("(p f) -> p f", p=P)[:, off:off + NT])
                    xs.append(xt)
                y = {}
                for c in range(C):
                    yt = tpool.tile([P, NT], F32, tag=f"y{c}")
                    nc.vector.tensor_scalar(
                        yt[:], xs[c][:], s_ap(c), mu_ap(c), ALU.mult, ALU.add)
                    y[c] = yt

                tr = tpool.tile([P, NT], F32, tag="tr")
                nc.vector.tensor_tensor(tr[:], y[0][:], y[3][:], ALU.add)
                tr2 = tpool.tile([P, NT], F32, tag="tr2")
                nc.vector.tensor_tensor(tr2[:], tr[:], y[5][:], ALU.add)
                q = tpool.tile([P, NT], F32, tag=f"q{name}")
                nc.vector.tensor_scalar(
                    q[:], tr2[:], 1.0 / 3.0, None, ALU.mult)

                c00 = tpool.tile([P, NT], F32, tag="c00")
                nc.vector.tensor_tensor(c00[:], y[0][:], q[:], ALU.subtract)
                c11 = tpool.tile([P, NT], F32, tag="c11")
                nc.vector.tensor_tensor(c11[:], y[3][:], q[:], ALU.subtract)
                s01 = tpool.tile([P, NT], F32, tag="s01")
                nc.vector.tensor_tensor(s01[:], c00[:], c11[:], ALU.add)

                # squares on ACT (Square is in every table set)
                sq = {}
                for nm, t in (("sq00", c00), ("sq11", c11), ("sq22", s01),
                              ("o1", y[1]), ("o2", y[2]), ("o3", y[4])):
                    st = tpool.tile([P, NT], F32, tag=nm)
                    nc.scalar.activation(st[:], t[:], AF.Square)
                    sq[nm] = st

                t1 = tpool.tile([P, NT], F32, tag="t1")
                nc.vector.tensor_tensor(t1[:], sq["sq00"][:], sq["sq11"][:], ALU.add)
                t2 = tpool.tile([P, NT], F32, tag="t2")
                nc.vector.tensor_tensor(t2[:], t1[:], sq["sq22"][:], ALU.add)
                t3 = tpool.tile([P, NT], F32, tag="t3")
                nc.vector.tensor_tensor(t3[:], sq["o1"][:], sq["o2"][:], ALU.add)
                t4 = tpool.tile([P, NT], F32, tag="t4")
                nc.vector.tensor_tensor(t4[:], t3[:], sq["o3"][:], ALU.add)
                t5 = tpool.tile([P, NT], F32, tag="t5")
                nc.vector.tensor_scalar(t5[:], t4[:], 2.0, None, ALU.mult)
                p2 = tpool.tile([P, NT], F32, tag=f"p2{name}")
                nc.vector.tensor_tensor(p2[:], t2[:], t5[:], ALU.add)

                # det(C) = s01*(o1 - c00*c11) - (c00*o3 + c11*o2) + 2*y1*y2*y4
                Pm = tpool.tile([P, NT], F32, tag="Pm")
                nc.vector.tensor_tensor(Pm[:], c00[:], c11[:], ALU.mult)
                K = tpool.tile([P, NT], F32, tag="K")
                nc.vector.tensor_tensor(K[:], sq["o1"][:], Pm[:], ALU.subtract)
                T1 = tpool.tile([P, NT], F32, tag="T1")
                nc.vector.tensor_tensor(T1[:], s01[:], K[:], ALU.mult)
                A_ = tpool.tile([P, NT], F32, tag="A_")
                nc.vector.tensor_tensor(A_[:], c00[:], sq["o3"][:], ALU.mult)
                B_ = tpool.tile([P, NT], F32, tag="B_")
                nc.vector.tensor_tensor(B_[:], c11[:], sq["o2"][:], ALU.mult)
                S_ = tpool.tile([P, NT], F32, tag="S_")
                nc.vector.tensor_tensor(S_[:], A_[:], B_[:], ALU.add)
                D_ = tpool.tile([P, NT], F32, tag="D_")
                nc.vector.tensor_tensor(D_[:], T1[:], S_[:], ALU.subtract)
                Y_ = tpool.tile([P, NT], F32, tag="Y_")
                nc.vector.tensor_tensor(Y_[:], y[1][:], y[2][:], ALU.mult)
                Y2 = tpool.tile([P, NT], F32, tag="Y2")
                nc.vector.tensor_tensor(Y2[:], Y_[:], y[4][:], ALU.mult)
                Z_ = tpool.tile([P, NT], F32, tag="Z_")
                nc.vector.tensor_scalar(Z_[:], Y2[:], 2.0, None, ALU.mult)
                det = tpool.tile([P, NT], F32, tag=f"det{name}")
                nc.vector.tensor_tensor(det[:], D_[:], Z_[:], ALU.add)

                # g = p2^3 - 54 det^2, clamped
                p2sq = tpool.tile([P, NT], F32, tag="p2sq")
                nc.scalar.activation(p2sq[:], p2[:], AF.Square)
                p2cu = tpool.tile([P, NT], F32, tag="p2cu")
                nc.vector.tensor_tensor(p2cu[:], p2sq[:], p2[:], ALU.mult)
                dsq = tpool.tile([P, NT], F32, tag="dsq")
                nc.scalar.activation(dsq[:], det[:], AF.Square, scale=SQRT54)
                g = tpool.tile([P, NT], F32, tag="g")
                nc.vector.tensor_tensor(g[:], p2cu[:], dsq[:], ALU.subtract)
                gc = tpool.tile([P, NT], F32, tag=f"gc{name}")
                nc.vector.tensor_scalar(gc[:], g[:], GMIN, None, ALU.max)
                rec = tpool.tile([P, NT], F32, tag=f"rec{name}")
                nc.vector.reciprocal(rec[:], gc[:])
                return {"p2": p2, "det": det, "rec": rec, "q": q}

            def trig_stage(st, name, p_scale):
                """sqrt ops then arctan/sin; returns (p, cs)."""
                p = tpool.tile([P, NT], F32, tag=f"p{name}")
                nc.scalar.activation(p[:], st["p2"][:], AF.Sqrt,
                                     bias=peps_ap, scale=p_scale)
                rsg = tpool.tile([P, NT], F32, tag=f"rsg{name}")
                nc.scalar.activation(rsg[:], st["rec"][:], AF.Sqrt)
                wk = tpool.tile([P, NT], F32, tag=f"wk{name}")
                nc.vector.tensor_tensor(wk[:], st["det"][:], rsg[:], ALU.mult)
                at = tpool.tile([P, NT], F32, tag=f"at{name}")
                nc.scalar.activation(at[:], wk[:], AF.Arctan, scale=SQRT54)
                cs = tpool.tile([P, NT], F32, tag=f"cs{name}")
                nc.scalar.activation(cs[:], at[:], AF.Sin,
                                     bias=pi3_ap, scale=1.0 / 3.0)
                return p, cs

            for _ in range(reps):
                for b in range(B):
                  for ch in range(NCH):
                    off = ch * NT
                    mt = iopool.tile([P, NT], I32, tag="mask")
                    nc.sync.dma_start(
                        mt[:], m_in[b].rearrange("(p f) -> p f", p=P)[:, off:off + NT])
                    mf = tpool.tile([P, NT], F32, tag="mf")
                    nc.vector.tensor_copy(mf[:], mt[:])

                    sti = tensor_chain(x_in, b, off, "i")
                    stt = tensor_chain(t_in, b, off, "t")
                    # batch the sqrt-set ops together, then the trig-set ops
                    p3, csi = trig_stage(sti, "i", 1.5)       # p3 = 3*p_in
                    p1, cst = trig_stage(stt, "t", 1.0 / 6.0)

                    u = tpool.tile([P, NT], F32, tag="u")
                    nc.vector.tensor_tensor(u[:], p3[:], csi[:], ALU.mult)
                    v = tpool.tile([P, NT], F32, tag="v")
                    nc.vector.tensor_tensor(v[:], p1[:], cst[:], ALU.mult)
                    w3 = tpool.tile([P, NT], F32, tag="w3")
                    nc.vector.tensor_tensor(w3[:], u[:], v[:], ALU.add)
                    diff = tpool.tile([P, NT], F32, tag="diff")
                    nc.vector.tensor_tensor(diff[:], w3[:], stt["q"][:],
                                            ALU.subtract)
                    dm = tpool.tile([P, NT], F32, tag="dm")
                    nc.vector.tensor_tensor(dm[:], diff[:], mf[:], ALU.mult)

                    adm = tpool.tile([P, NT], F32, tag="adm")
                    asum = ppool.tile([P, 1], F32, tag="asum")
                    nc.scalar.activation(adm[:], dm[:], AF.Abs,
                                         accum_out=asum[:])
                    nc.vector.tensor_tensor(lacc[:], lacc[:], asum[:], ALU.add)

                    msum = ppool.tile([P, 1], F32, tag="msum")
                    nc.vector.tensor_reduce(msum[:], mf[:],
                                            mybir.AxisListType.X, ALU.add)
                    nc.vector.tensor_tensor(cacc[:], cacc[:], msum[:], ALU.add)

            res = apool.tile([P, 2], F32, tag="res")
            nc.vector.tensor_copy(res[:, 0:1], lacc[:])
            nc.vector.tensor_copy(res[:, 1:2], cacc[:])
            nc.sync.dma_start(out[:], res[:])

    nc.compile()
    return nc


def get_module(reps: int = 1):
    if reps not in _CACHE:
        _CACHE[reps] = _build(reps)
    return _CACHE[reps]


def make_in_maps(input_data, target, mask, gt_mean, gt_std):
    xs = np.ascontiguousarray(input_data.reshape(B, C, HWD))
    ts = np.ascontiguousarray(target.reshape(B, C, HWD))
    ms = np.ascontiguousarray(mask.reshape(B, HWD))
    scal = np.zeros((P, 16), np.float32)
    scal[:, 0:6] = np.asarray(gt_std, np.float32).reshape(1, 6)
    scal[:, 6:12] = np.asarray(gt_mean, np.float32).reshape(1, 6)
    scal[:, 12] = PEPS
    scal[:, 13] = PI3
    in_maps = []
    for k in range(N_CORES):
        sl = slice(k * SH, (k + 1) * SH)
        in_maps.append({
            "input_data": np.ascontiguousarray(xs[:, :, sl]),
            "target": np.ascontiguousarray(ts[:, :, sl]),
            "mask": np.ascontiguousarray(ms[:, sl]),
            "scal": scal,
        })
    return in_maps


def kernel(input_data, target, mask, gt_mean, gt_std):
    nc = get_module()
    in_maps = make_in_maps(input_data, target, mask, gt_mean, gt_std)
    r = run_bass_kernel_spmd(nc, in_maps, core_ids=list(range(N_CORES)))
    s = 0.0
    n = 0.0
    for i in range(N_CORES):
        o = r.results[i]["out"].astype(np.float64)
        s += o[:, 0].sum()
        n += o[:, 1].sum()
    loss = s / max(n, 1.0)
    return np.float32(loss)


# revision 5
# speedup vs baseline: 51.0908x; 51.0908x over previous
"""Trainium2 Bass kernel for the ANI (anisotropy) L1 loss.

Math (per voxel, per 3x3 symmetric tensor with channels xx,xy,xz,yy,yz,zz):
  y_c = gt_std[c] * x_c + gt_mean[c]            (affine; mask applied at the end)
  A   = [[y0,y1,y2],[y1,y3,y4],[y2,y4,y5]]
  q   = tr(A)/3 ;  C = A - q I
  p2  = ||C||_F^2 ;  p = sqrt(p2/6) ;  det = det(C)
  r   = det / (2 p^3) in [-1,1] ;  phi = arccos(r)/3
  ani(input)  = 3 p cos(phi)          (= l2 - (l0+l1)/2)
  ani(target) = q - p cos(phi)        (= (l0+l1)/2)
  loss = sum(|ani_in - ani_tg| * m) / max(sum(m), 1)

On-device identities (HW tables lack arccos/cos/rsqrt):
  cos(arccos(r)/3) = sin(pi/3 + arctan(w)/3)
  w = r/sqrt(1-r^2) = sqrt(54) * det / sqrt(g),  g = p2^3 - 54 det^2
(no normalization by p^3 ever happens; g is computed by one fused custom
DVE op with a floor clamp, then 1/g via RECIPROCAL_APPROX_FAST and sqrt on ACT).

Sharding: pure data-parallel over the flattened spatial axis (8 cores); each
core returns per-partition (masked |diff| sum, mask count) partials and the
host reduces them to the scalar loss.
"""

import numpy as np

import concourse.bass as bass
import concourse.tile as tile
from concourse import bacc, mybir
from concourse.bass_utils import run_bass_kernel_spmd

F32 = mybir.dt.float32
BF16 = mybir.dt.bfloat16
I32 = mybir.dt.int32
ALU = mybir.AluOpType
AF = mybir.ActivationFunctionType

N_CORES = 8
B, C = 4, 6
HWD = 96 * 96 * 96          # 884736
SH = HWD // N_CORES         # 110592
BSH = B * SH                # 442368 voxels per core
P = 128
FREE = BSH // P             # 3456
NT = 1728                   # free elems per tile
NCH = FREE // NT            # chunks

SQRT54 = float(np.sqrt(54.0))
GMIN = 1e-30
PEPS = 1e-25
PI3 = float(np.pi / 3.0)

_CACHE = {}

# bisect/config flags
USE_CUSTOM_GCLAMP = True
USE_FAST_RECIP = True

# ---------------------------------------------------------------------------
# Custom fused DVE op:  gc = max(p2^3 - 54*det^2, GMIN)
# ---------------------------------------------------------------------------
_GCLAMP = None


def _register_gclamp():
    global _GCLAMP
    if _GCLAMP is not None:
        return _GCLAMP
    import concourse.dve_ops as dve_ops
    from concourse.dve_ops import DveOp
    from concourse.dve_spec import Spec, Src0, Src1, C0, C2, maxx, sq, lower, _has_src1
    from concourse.dve_uop import DveOpSpec

    name = "ANI_GCLAMP"
    body = maxx((sq(Src0) * Src0) - (sq(Src1) * C2), C0)

    def ref(in0, in1, c0, c1, c2):
        x = in0.astype(np.float32)
        d = in1.astype(np.float32)
        return np.maximum(x * x * x - d * d * c2, c0)

    spec = Spec(body=body, reference=ref)
    row = dve_ops._CUSTOM_DVE_ROW_BASE + len(dve_ops.OPS)
    tmp = DveOpSpec(name=name, opcode=row, uops=lower(spec, ver="v3"),
                    rd1_en=_has_src1(spec))
    op = DveOp(name, spec, subdim=False, uops_sha={"v3": tmp.sha("v3")})
    dve_ops.OPS.append(op)
    dve_ops.CUSTOM_DVE_SPECS[name] = spec
    dve_ops._SUB_OPCODE_FOR_NAME[name] = row
    _GCLAMP = op
    return op


def _build(reps: int = 1):
    gclamp = _register_gclamp() if USE_CUSTOM_GCLAMP else None
    nc = bacc.Bacc("TRN2", target_bir_lowering=False, debug=False,
                   num_devices=N_CORES)
    x_in = nc.dram_tensor("input_data", [C, BSH], F32, kind="ExternalInput")
    t_in = nc.dram_tensor("target", [C, BSH], F32, kind="ExternalInput")
    m_in = nc.dram_tensor("mask", [BSH], I32, kind="ExternalInput")
    sc_in = nc.dram_tensor("scal", [P, 16], F32, kind="ExternalInput")
    out = nc.dram_tensor("out", [P, 2], F32, kind="ExternalOutput")

    with tile.TileContext(nc) as tc:
        with (
            tc.tile_pool(name="const", bufs=1) as cpool,
            tc.tile_pool(name="io", bufs=2) as iopool,
            tc.tile_pool(name="tmp", bufs=1) as tpool,
            tc.tile_pool(name="acc", bufs=1) as apool,
            tc.tile_pool(name="part", bufs=2) as ppool,
        ):
            scal = cpool.tile([P, 16], F32, tag="scal")
            nc.sync.dma_start(scal[:], sc_in[:])
            lacc = apool.tile([P, 1], F32, tag="lacc")
            cacc = apool.tile([P, 1], F32, tag="cacc")
            nc.vector.memset(lacc[:], 0.0)
            nc.vector.memset(cacc[:], 0.0)

            def s_ap(c):
                return scal[:, c:c + 1]

            def mu_ap(c):
                return scal[:, 6 + c:7 + c]

            peps_ap = scal[:, 12:13]
            pi3_ap = scal[:, 13:14]

            def tensor_chain(src, off, name):
                """Compute (p2, det, rec=1/gc, q) tiles for one tensor.
                Scratch (y*, s*) tags are shared between input/target phases."""
                xs = []
                for c in range(C):
                    xt = iopool.tile([P, NT], F32, tag=f"x{c}")
                    nc.sync.dma_start(
                        xt[:],
                        src[c].rearrange("(p f) -> p f", p=P)[:, off:off + NT])
                    xs.append(xt)
                # y tiles (bf16); c00/c11/s01 overwrite y0/y3/y5 in place
                y = []
                for c in range(C):
                    yt = tpool.tile([P, NT], BF16, tag=f"y{c}")
                    nc.vector.tensor_scalar(
                        yt[:], xs[c][:], s_ap(c), mu_ap(c), ALU.mult, ALU.add)
                    y.append(yt)

                s1 = tpool.tile([P, NT], BF16, tag="s1")   # tr
                nc.vector.tensor_tensor(s1[:], y[0][:], y[3][:], ALU.add)
                s2 = tpool.tile([P, NT], BF16, tag="s2")   # tr2
                nc.vector.tensor_tensor(s2[:], s1[:], y[5][:], ALU.add)
                q = tpool.tile([P, NT], BF16, tag=f"q{name}")
                nc.vector.tensor_scalar(q[:], s2[:], 1.0 / 3.0, None, ALU.mult)

                c00 = y[0]
                nc.vector.tensor_tensor(c00[:], y[0][:], q[:], ALU.subtract)
                c11 = y[3]
                nc.vector.tensor_tensor(c11[:], y[3][:], q[:], ALU.subtract)
                s01 = y[5]                                  # s01 = -c22 = q - y5
                nc.vector.tensor_tensor(s01[:], q[:], y[5][:], ALU.subtract)

                # squares (ACT; Square lives in every table set)
                sq00 = tpool.tile([P, NT], BF16, tag="s3")
                nc.scalar.activation(sq00[:], c00[:], AF.Square)
                sq11 = tpool.tile([P, NT], BF16, tag="s4")
                nc.scalar.activation(sq11[:], c11[:], AF.Square)
                sq22 = tpool.tile([P, NT], BF16, tag="s5")
                nc.scalar.activation(sq22[:], s01[:], AF.Square)
                o1 = tpool.tile([P, NT], BF16, tag="s6")
                nc.scalar.activation(o1[:], y[1][:], AF.Square)
                o2 = tpool.tile([P, NT], BF16, tag="s7")
                nc.scalar.activation(o2[:], y[2][:], AF.Square)
                o3 = tpool.tile([P, NT], BF16, tag="s8")
                nc.scalar.activation(o3[:], y[4][:], AF.Square)

                # p2 = sq00+sq11+sq22 + 2*(o1+o2+o3); chains in place
                nc.vector.tensor_tensor(sq00[:], sq00[:], sq11[:], ALU.add)
                nc.vector.tensor_tensor(sq00[:], sq00[:], sq22[:], ALU.add)
                nc.vector.tensor_tensor(s2[:], o1[:], o2[:], ALU.add)
                nc.vector.tensor_tensor(s2[:], s2[:], o3[:], ALU.add)
                nc.vector.tensor_scalar(s2[:], s2[:], 2.0, None, ALU.mult)
                p2 = tpool.tile([P, NT], BF16, tag=f"p2{name}")
                nc.vector.tensor_tensor(p2[:], sq00[:], s2[:], ALU.add)

                # det = s01*(o1 - c00*c11) - (c00*o3 + c11*o2) + 2*y1*y2*y4
                nc.vector.tensor_tensor(sq11[:], c00[:], c11[:], ALU.mult)       # Pm
                nc.vector.tensor_tensor(sq11[:], o1[:], sq11[:], ALU.subtract)   # K
                nc.vector.tensor_tensor(sq11[:], s01[:], sq11[:], ALU.mult)      # T1
                nc.vector.tensor_tensor(o3[:], c00[:], o3[:], ALU.mult)          # A_
                nc.vector.tensor_tensor(o2[:], c11[:], o2[:], ALU.mult)          # B_
                nc.vector.tensor_tensor(o3[:], o3[:], o2[:], ALU.add)            # S_
                nc.vector.tensor_tensor(sq11[:], sq11[:], o3[:], ALU.subtract)   # D_
                nc.vector.tensor_tensor(y[1][:], y[1][:], y[2][:], ALU.mult)     # Y
                nc.vector.tensor_tensor(y[1][:], y[1][:], y[4][:], ALU.mult)     # Y2
                nc.vector.tensor_scalar(y[1][:], y[1][:], 2.0, None, ALU.mult)
                det = tpool.tile([P, NT], BF16, tag=f"det{name}")
                nc.vector.tensor_tensor(det[:], sq11[:], y[1][:], ALU.add)

                # gc = max(p2^3 - 54 det^2, GMIN) fused; rec = 1/gc
                gc = tpool.tile([P, NT], F32, tag=f"gc{name}")
                if USE_CUSTOM_GCLAMP:
                    nc.vector._custom_dve(gclamp, out=gc[:], in0=p2[:],
                                          in1=det[:], s0=GMIN, imm2=54.0)
                else:
                    p2sq = tpool.tile([P, NT], F32, tag="p2sq")
                    nc.scalar.activation(p2sq[:], p2[:], AF.Square)
                    nc.vector.tensor_tensor(p2sq[:], p2sq[:], p2[:], ALU.mult)
                    dsq = tpool.tile([P, NT], F32, tag="dsq")
                    nc.scalar.activation(dsq[:], det[:], AF.Square, scale=SQRT54)
                    nc.vector.tensor_tensor(gc[:], p2sq[:], dsq[:], ALU.subtract)
                    nc.vector.tensor_scalar(gc[:], gc[:], GMIN, None, ALU.max)
                if USE_FAST_RECIP:
                    nc.vector.reciprocal_approx_fast(gc[:], gc[:])
                else:
                    rec2 = tpool.tile([P, NT], F32, tag=f"rec{name}")
                    nc.vector.reciprocal(rec2[:], gc[:])
                    gc = rec2
                return {"p2": p2, "det": det, "rec": gc, "q": q}

            for _ in range(reps):
                for ch in range(NCH):
                    off = ch * NT
                    mt = iopool.tile([P, NT], I32, tag="mask")
                    nc.sync.dma_start(
                        mt[:],
                        m_in.rearrange("(p f) -> p f", p=P)[:, off:off + NT])
                    mf = tpool.tile([P, NT], BF16, tag="mf")
                    nc.vector.tensor_copy(mf[:], mt[:])

                    sti = tensor_chain(x_in, off, "i")
                    stt = tensor_chain(t_in, off, "t")

                    # ---- sqrt-set batch ----
                    p3 = sti["p2"]          # in place: p3 = sqrt(1.5*p2+eps)
                    nc.scalar.activation(p3[:], sti["p2"][:], AF.Sqrt,
                                         bias=peps_ap, scale=1.5)
                    p1 = stt["p2"]
                    nc.scalar.activation(p1[:], stt["p2"][:], AF.Sqrt,
                                         bias=peps_ap, scale=1.0 / 6.0)
                    rsgi = tpool.tile([P, NT], BF16, tag="s1")
                    nc.scalar.activation(rsgi[:], sti["rec"][:], AF.Sqrt)
                    rsgt = tpool.tile([P, NT], BF16, tag="s2")
                    nc.scalar.activation(rsgt[:], stt["rec"][:], AF.Sqrt)

                    # ---- wk = det * rsqrt(g) (DVE) ----
                    wki = sti["det"]
                    nc.vector.tensor_tensor(wki[:], sti["det"][:], rsgi[:],
                                            ALU.mult)
                    wkt = stt["det"]
                    nc.vector.tensor_tensor(wkt[:], stt["det"][:], rsgt[:],
                                            ALU.mult)

                    # ---- trig-set batch ----
                    nc.scalar.activation(wki[:], wki[:], AF.Arctan, scale=SQRT54)
                    nc.scalar.activation(wki[:], wki[:], AF.Sin,
                                         bias=pi3_ap, scale=1.0 / 3.0)  # cs_i
                    nc.scalar.activation(wkt[:], wkt[:], AF.Arctan, scale=SQRT54)
                    nc.scalar.activation(wkt[:], wkt[:], AF.Sin,
                                         bias=pi3_ap, scale=1.0 / 3.0)  # cs_t

                    # ---- final: |3 p_i cs_i + p_t cs_t - q_t| * m ----
                    nc.vector.tensor_tensor(wki[:], p3[:], wki[:], ALU.mult)   # u
                    nc.vector.tensor_tensor(wkt[:], p1[:], wkt[:], ALU.mult)   # v
                    nc.vector.tensor_tensor(wki[:], wki[:], wkt[:], ALU.add)   # w3
                    nc.vector.tensor_tensor(wki[:], wki[:], stt["q"][:],
                                            ALU.subtract)                      # diff
                    nc.vector.tensor_tensor(wki[:], wki[:], mf[:], ALU.mult)   # dm

                    asum = ppool.tile([P, 1], F32, tag="asum")
                    nc.scalar.activation(wki[:], wki[:], AF.Abs,
                                         accum_out=asum[:])
                    nc.vector.tensor_tensor(lacc[:], lacc[:], asum[:], ALU.add)
                    msum = ppool.tile([P, 1], F32, tag="msum")
                    nc.vector.tensor_reduce(msum[:], mf[:],
                                            mybir.AxisListType.X, ALU.add)
                    nc.vector.tensor_tensor(cacc[:], cacc[:], msum[:], ALU.add)

            res = apool.tile([P, 2], F32, tag="res")
            nc.vector.tensor_copy(res[:, 0:1], lacc[:])
            nc.vector.tensor_copy(res[:, 1:2], cacc[:])
            nc.sync.dma_start(out[:], res[:])

    nc.compile()
    return nc


def get_module(reps: int = 1):
    if reps not in _CACHE:
        _CACHE[reps] = _build(reps)
    return _CACHE[reps]


def make_in_maps(input_data, target, mask, gt_mean, gt_std):
    xs = np.asarray(input_data).reshape(B, C, HWD)
    ts = np.asarray(target).reshape(B, C, HWD)
    ms = np.asarray(mask).reshape(B, HWD)
    scal = np.zeros((P, 16), np.float32)
    scal[:, 0:6] = np.asarray(gt_std, np.float32).reshape(1, 6)
    scal[:, 6:12] = np.asarray(gt_mean, np.float32).reshape(1, 6)
    scal[:, 12] = PEPS
    scal[:, 13] = PI3
    in_maps = []
    for k in range(N_CORES):
        sl = slice(k * SH, (k + 1) * SH)
        in_maps.append({
            "input_data": np.ascontiguousarray(
                xs[:, :, sl].transpose(1, 0, 2)).reshape(C, BSH),
            "target": np.ascontiguousarray(
                ts[:, :, sl].transpose(1, 0, 2)).reshape(C, BSH),
            "mask": np.ascontiguousarray(ms[:, sl]).reshape(BSH),
            "scal": scal,
        })
    return in_maps


def kernel(input_data, target, mask, gt_mean, gt_std):
    nc = get_module()
    in_maps = make_in_maps(input_data, target, mask, gt_mean, gt_std)
    r = run_bass_kernel_spmd(nc, in_maps, core_ids=list(range(N_CORES)))
    s = 0.0
    n = 0.0
    for i in range(N_CORES):
        o = r.results[i]["out"].astype(np.float64)
        s += o[:, 0].sum()
        n += o[:, 1].sum()
    return np.float32(s / max(n, 1.0))
